# revision 1
# baseline (speedup 1.0000x reference)
"""GroupedQueryAttention (B=1, T=2048, D=4096, 32 q-heads / 8 kv-heads, hd=128)
on 8 trn2 NeuronCores.

Sharding: kv-head parallel — core c owns kv head c and its 4 query heads.
Each core: QKV projections (x.T streamed once), rope, causal attention in
transposed [k, q] score layout (softmax without max-subtraction: fp32 exp
can't overflow at these score magnitudes), AllGather of y.T, then a
column-parallel wo matmul so no AllReduce is needed.
"""
import sys

sys.path.insert(0, "/opt/trn_rl_repo")

import numpy as np

import concourse.bacc as bacc
import concourse.tile as tile
from concourse import mybir
from concourse.bass_utils import run_bass_kernel_spmd
from concourse.masks import make_identity

N_CORES = 8
T = 2048
DIM = 4096
HD = 128
NH = 32
NKV = 8
NREP = NH // NKV  # 4 query heads per core
NCHUNK = T // 512  # 4 chunks of 512 along T
NKT = DIM // 128  # 32 contraction tiles for the projections
NTT = T // 128  # 16 row tiles for the wo matmul
F32 = mybir.dt.float32
SCALE = 1.0 / float(np.sqrt(HD))

_cached = {}


import os

DEBUG = bool(int(os.environ.get("GQA_DEBUG", "0")))
MAXPHASE = int(os.environ.get("GQA_MAXPHASE", "4"))


def _build_kernel():
    if "nc" in _cached:
        return _cached["nc"]

    nc = bacc.Bacc("TRN2", target_bir_lowering=False)

    xT = nc.dram_tensor("xT", [DIM, T], F32, kind="ExternalInput")
    cos2 = nc.dram_tensor("cos2", [128, T], F32, kind="ExternalInput")
    sin2 = nc.dram_tensor("sin2", [128, T], F32, kind="ExternalInput")
    masks = nc.dram_tensor("masks", [128, 4 * 512], F32, kind="ExternalInput")
    wqT = nc.dram_tensor("wqT", [DIM, NREP * HD], F32, kind="ExternalInput")
    wkT = nc.dram_tensor("wkT", [DIM, HD], F32, kind="ExternalInput")
    wvT = nc.dram_tensor("wvT", [DIM, HD], F32, kind="ExternalInput")
    woT = nc.dram_tensor("woT", [DIM, NREP * HD], F32, kind="ExternalInput")
    out = nc.dram_tensor("out", [T, NREP * HD], F32, kind="ExternalOutput")

    if DEBUG:
        dbg_q = nc.dram_tensor("dbg_q", [128, NREP * T], F32, kind="ExternalOutput")
        dbg_k = nc.dram_tensor("dbg_k", [128, T], F32, kind="ExternalOutput")
        dbg_v = nc.dram_tensor("dbg_v", [128, NTT * HD], F32, kind="ExternalOutput")
        dbg_y = nc.dram_tensor("dbg_y", [NREP * HD, T], F32, kind="ExternalOutput")

    y_in = nc.dram_tensor("y_in", [NREP * HD, T], F32, kind="Internal")
    y_all = nc.dram_tensor(
        "y_all", [DIM, T], F32, kind="Internal", addr_space="Shared"
    )

    with tile.TileContext(nc) as tc:
        with (
            tc.tile_pool(name="consts", bufs=1) as consts,
            tc.tile_pool(name="weights", bufs=1) as weights,
            tc.tile_pool(name="acts", bufs=1) as acts,
            tc.tile_pool(name="stream", bufs=3) as stream,
            tc.tile_pool(name="work", bufs=2) as work,
            tc.tile_pool(name="expp", bufs=4) as expp,
            tc.tile_pool(name="outp", bufs=3) as outp,
            tc.tile_pool(name="psum", bufs=8, space="PSUM") as psum,
        ):
            # ---------- constants ----------
            cos_sb = consts.tile([128, T], F32, tag="cos")
            nc.sync.dma_start(out=cos_sb, in_=cos2[:, :])
            sin_sb = consts.tile([128, T], F32, tag="sin")
            nc.sync.dma_start(out=sin_sb, in_=sin2[:, :])
            mask_sb = consts.tile([128, 4 * 512], F32, tag="mask")
            nc.sync.dma_start(out=mask_sb, in_=masks[:, :])
            ones_col = consts.tile([128, 1], F32, tag="onesc")
            nc.vector.memset(ones_col, 1.0)
            ones_row = consts.tile([1, 128], F32, tag="onesr")
            nc.vector.memset(ones_row, 1.0)
            ident = consts.tile([128, 128], F32, tag="ident")
            make_identity(nc, ident)

            # ---------- resident weights (wq now; wo reuses the slot later) ----------
            wq_sb = weights.tile([128, NKT, NREP * HD], F32, tag="wbig")
            nc.sync.dma_start(
                out=wq_sb, in_=wqT.rearrange("(n p) m -> p n m", p=128)
            )

            # activations that live through the attention phase
            qT_sb = acts.tile([128, NREP, T], F32, tag="qt")
            kT_sb = acts.tile([128, T], F32, tag="kt")
            vkd_sb = acts.tile([128, NTT, HD], F32, tag="vkd")

            # ---------- phase 1: QKV projections + rope, chunk by chunk ----------
            for qc in range(NCHUNK):
                cs = slice(512 * qc, 512 * (qc + 1))
                q_ps = [
                    psum.tile([128, 512], F32, tag="bank", name=f"qps{qc}_{h}")
                    for h in range(NREP)
                ]
                k_ps = psum.tile([128, 512], F32, tag="bank")
                v_ps = psum.tile([128, 512], F32, tag="bank")
                for kt in range(NKT):
                    xt = stream.tile([128, 512], F32, tag="xt")
                    nc.sync.dma_start(
                        out=xt, in_=xT[128 * kt:128 * (kt + 1), cs]
                    )
                    wk_t = stream.tile([128, HD], F32, tag="wk")
                    nc.sync.dma_start(
                        out=wk_t, in_=wkT[128 * kt:128 * (kt + 1), :]
                    )
                    wv_t = stream.tile([128, HD], F32, tag="wv")
                    nc.sync.dma_start(
                        out=wv_t, in_=wvT[128 * kt:128 * (kt + 1), :]
                    )
                    st = kt == 0
                    sp = kt == NKT - 1
                    for h in range(NREP):
                        nc.tensor.matmul(
                            q_ps[h],
                            lhsT=wq_sb[:, kt, 128 * h:128 * (h + 1)],
                            rhs=xt,
                            start=st,
                            stop=sp,
                        )
                    nc.tensor.matmul(k_ps, lhsT=wk_t, rhs=xt, start=st, stop=sp)
                    nc.tensor.matmul(v_ps, lhsT=wv_t, rhs=xt, start=st, stop=sp)

                # v computed in [hd, T] layout; transpose 128x128 blocks to [k, hd]
                v_sb = work.tile([128, 512], F32, tag="vsb")
                nc.scalar.copy(v_sb, v_ps)
                for s in range(4):
                    vt_ps = psum.tile([128, 128], F32, tag="bank", name=f"vt{qc}_{s}")
                    nc.tensor.transpose(vt_ps, v_sb[:, 128 * s:128 * (s + 1)], ident)
                    nc.scalar.copy(vkd_sb[:, 4 * qc + s, :], vt_ps)

                # rope for the 4 q heads and k
                for h in range(NREP + 1):
                    p = q_ps[h] if h < NREP else k_ps
                    dst = qT_sb[:, h, cs] if h < NREP else kT_sb[:, cs]
                    sw = work.tile([128, 512], F32, tag="sw")
                    nc.scalar.copy(sw[0:64, :], p[64:128, :])
                    nc.scalar.copy(sw[64:128, :], p[0:64, :])
                    # dst = p * cos + sw * (+-sin)
                    nc.vector.tensor_mul(dst, p, cos_sb[:, cs])
                    nc.vector.tensor_mul(sw, sw, sin_sb[:, cs])
                    nc.vector.tensor_add(dst, dst, sw)

            # ---------- phase 2: causal attention, transposed score layout ----------
            for h in range(NREP if MAXPHASE >= 2 else 0):
                for qc in range(NCHUNK):
                    cs = slice(512 * qc, 512 * (qc + 1))
                    nkt = 4 * qc + 4  # causal: k tiles 0 .. 4*qc+3
                    yT_ps = psum.tile([128, 512], F32, tag="bank")
                    l_acc = work.tile([128, 512], F32, tag="lacc")
                    nc.vector.memset(l_acc, 0.0)
                    for kt in range(nkt):
                        sT_ps = psum.tile([128, 512], F32, tag="bank")
                        nc.tensor.matmul(
                            sT_ps,
                            lhsT=kT_sb[:, 128 * kt:128 * (kt + 1)],
                            rhs=qT_sb[:, h, cs],
                            start=True,
                            stop=True,
                        )
                        e_sb = expp.tile([128, 512], F32, tag="exp")
                        nc.scalar.activation(
                            e_sb, sT_ps, mybir.ActivationFunctionType.Exp,
                            scale=SCALE,
                        )
                        d = kt - 4 * qc
                        if d >= 0:  # diagonal block: zero the k > q half
                            nc.vector.tensor_mul(
                                e_sb, e_sb, mask_sb[:, 512 * d:512 * (d + 1)]
                            )
                        nc.vector.tensor_add(l_acc, l_acc, e_sb)
                        nc.tensor.matmul(
                            yT_ps,
                            lhsT=vkd_sb[:, kt, :],
                            rhs=e_sb,
                            start=(kt == 0),
                            stop=(kt == nkt - 1),
                        )
                    # softmax denominator -> reciprocal -> broadcast to 128 parts
                    l_ps = psum.tile([128, 512], F32, tag="bank")
                    nc.tensor.matmul(
                        l_ps[0:1, :], lhsT=ones_col[:, 0:1], rhs=l_acc,
                        start=True, stop=True,
                    )
                    recip = work.tile([1, 512], F32, tag="recip")
                    nc.vector.reciprocal(recip, l_ps[0:1, :])
                    bc_ps = psum.tile([128, 512], F32, tag="bank")
                    nc.tensor.matmul(
                        bc_ps, lhsT=ones_row[0:1, :], rhs=recip[0:1, :],
                        start=True, stop=True,
                    )
                    bc_sb = work.tile([128, 512], F32, tag="bc")
                    nc.scalar.copy(bc_sb, bc_ps)
                    yn_sb = work.tile([128, 512], F32, tag="yn")
                    nc.vector.tensor_mul(yn_sb, yT_ps, bc_sb)
                    nc.sync.dma_start(
                        out=y_in[128 * h:128 * (h + 1), cs], in_=yn_sb
                    )

            if DEBUG:
                nc.sync.dma_start(
                    out=dbg_q.rearrange("p (n m) -> p n m", n=NREP), in_=qT_sb
                )
                nc.sync.dma_start(out=dbg_k[:, :], in_=kT_sb)
                nc.sync.dma_start(
                    out=dbg_v.rearrange("p (n m) -> p n m", n=NTT), in_=vkd_sb
                )
                nc.sync.dma_start(out=dbg_y[:, :], in_=y_in[:, :])

            # ---------- phase 3: AllGather y.T across the 8 cores ----------
            nc.gpsimd.collective_compute(
                "AllGather",
                mybir.AluOpType.bypass,
                ins=[y_in[:, :]],
                outs=[y_all[:, :]],
                replica_groups=[list(range(N_CORES))],
            )

            # ---------- phase 4: out = y @ wo_c.T (column-parallel) ----------
            wo_sb = weights.tile([128, NKT, NREP * HD], F32, tag="wbig")
            nc.sync.dma_start(
                out=wo_sb, in_=woT.rearrange("(n p) m -> p n m", p=128)
            )
            y_r = y_all.rearrange("(n p) m -> p n m", p=128)
            for tt in range(NTT):
                # alternate between two dead slots for double buffering
                ytag = "qt" if tt % 2 == 0 else "kt"
                y_t = acts.tile([128, NKT, 128], F32, tag=ytag)
                nc.sync.dma_start(
                    out=y_t, in_=y_r[:, :, 128 * tt:128 * (tt + 1)]
                )
                o_ps = psum.tile([128, 512], F32, tag="bank")
                for kt in range(NKT):
                    nc.tensor.matmul(
                        o_ps,
                        lhsT=y_t[:, kt, :],
                        rhs=wo_sb[:, kt, :],
                        start=(kt == 0),
                        stop=(kt == NKT - 1),
                    )
                o_sb = outp.tile([128, 512], F32, tag="osb")
                nc.scalar.copy(o_sb, o_ps)
                nc.sync.dma_start(
                    out=out[128 * tt:128 * (tt + 1), :], in_=o_sb
                )

    nc.compile()
    _cached["nc"] = nc
    return nc


def _build_in_maps(inputs):
    return _shard_inputs(**inputs)


def _shard_inputs(x, cos, sin, wq, wk, wv, wo, start_pos):
    x = np.asarray(x, dtype=np.float32)
    cos = np.asarray(cos, dtype=np.float32)
    sin = np.asarray(sin, dtype=np.float32)
    wq = np.asarray(wq, dtype=np.float32)
    wk = np.asarray(wk, dtype=np.float32)
    wv = np.asarray(wv, dtype=np.float32)
    wo = np.asarray(wo, dtype=np.float32)
    sp = int(start_pos)

    xT = np.ascontiguousarray(x[0].T)  # (DIM, T)
    cosT = np.ascontiguousarray(cos[sp:sp + T].T)  # (64, T)
    sinT = np.ascontiguousarray(sin[sp:sp + T].T)
    cos2 = np.concatenate([cosT, cosT], axis=0)  # (128, T)
    sin2 = np.concatenate([-sinT, sinT], axis=0)  # rotate-half signs folded in

    kk = np.arange(128)[:, None]
    qq = np.arange(512)[None, :]
    masks = np.concatenate(
        [(kk + 128 * d <= qq).astype(np.float32) for d in range(4)], axis=1
    )  # (128, 2048)

    in_maps = []
    for c in range(N_CORES):
        qrows = slice(NREP * HD * c, NREP * HD * (c + 1))
        krows = slice(HD * c, HD * (c + 1))
        in_maps.append({
            "xT": xT,
            "cos2": cos2,
            "sin2": sin2,
            "masks": masks,
            "wqT": np.ascontiguousarray(wq[qrows, :].T),
            "wkT": np.ascontiguousarray(wk[krows, :].T),
            "wvT": np.ascontiguousarray(wv[krows, :].T),
            "woT": np.ascontiguousarray(wo[qrows, :].T),
        })
    return in_maps


def kernel(x, cos, sin, wq, wk, wv, wo, start_pos):
    in_maps = _shard_inputs(x, cos, sin, wq, wk, wv, wo, start_pos)
    nc = _build_kernel()
    res = run_bass_kernel_spmd(nc, in_maps, core_ids=list(range(N_CORES)))
    out = np.concatenate([res.results[c]["out"] for c in range(N_CORES)], axis=1)
    return out.reshape(1, T, DIM)



# revision 8
# speedup vs baseline: 3.0278x; 3.0278x over previous
"""GroupedQueryAttention (B=1, T=2048, D=4096, 32 q-heads / 8 kv-heads, hd=128)
on 8 trn2 NeuronCores.

Sharding: kv-head parallel — core c owns kv head c and its 4 query heads.
v3: 16-bit matmuls (fp16 on the q/k score path and projections for mantissa,
bf16 on the exp/value path for range), chunk-pipelined schedule with one
AllGather per 512-token chunk overlapped two chunks deep, wo matmul
column-parallel so no AllReduce is needed.  Causal attention in transposed
[k, q] score layout (softmax without max-subtraction: fp32 exp can't
overflow at these score magnitudes).
"""
import sys

sys.path.insert(0, "/opt/trn_rl_repo")

import numpy as np

import concourse.bacc as bacc
import concourse.tile as tile
from concourse import mybir
from concourse.bass_utils import run_bass_kernel_spmd
from concourse.masks import make_identity

N_CORES = 8
T = 2048
DIM = 4096
HD = 128
NH = 32
NKV = 8
NREP = NH // NKV  # 4 query heads per core
NCHUNK = T // 512  # 4 chunks of 512 along T
NKT = DIM // 128  # 32 contraction tiles for the projections
NTT = T // 128  # 16 row tiles for the wo matmul
F32 = mybir.dt.float32
FP16 = mybir.dt.float16
BF16 = mybir.dt.bfloat16
SCALE = 1.0 / float(np.sqrt(HD))

_cached = {}


def _build_kernel():
    if "nc" in _cached:
        return _cached["nc"]

    nc = bacc.Bacc("TRN2", target_bir_lowering=False)

    xT = nc.dram_tensor("xT", [DIM, T], FP16, kind="ExternalInput")
    cos2 = nc.dram_tensor("cos2", [128, T], F32, kind="ExternalInput")
    sin2 = nc.dram_tensor("sin2", [128, T], F32, kind="ExternalInput")
    masks = nc.dram_tensor("masks", [128, 4 * 512], BF16, kind="ExternalInput")
    # weights pre-packed on host: [128, n*m] with partition-contiguous rows
    wq_p = nc.dram_tensor("wq_p", [128, NKT * NREP * HD], FP16, kind="ExternalInput")
    wk_p = nc.dram_tensor("wk_p", [128, NKT * HD], FP16, kind="ExternalInput")
    wv_p = nc.dram_tensor("wv_p", [128, NKT * HD], FP16, kind="ExternalInput")
    wo_p = nc.dram_tensor("wo_p", [128, NKT * NREP * HD], FP16, kind="ExternalInput")
    out = nc.dram_tensor("out", [T, NREP * HD], F32, kind="ExternalOutput")

    y_in = [
        nc.dram_tensor(f"y_in{qc}", [NREP * HD, 512], FP16, kind="Internal")
        for qc in range(NCHUNK)
    ]
    y_all = [
        nc.dram_tensor(
            f"y_all{qc}", [DIM, 512], FP16, kind="Internal", addr_space="Shared"
        )
        for qc in range(NCHUNK)
    ]

    with tile.TileContext(nc) as tc:
        with (
            tc.tile_pool(name="consts", bufs=1) as consts,
            tc.tile_pool(name="weights", bufs=1) as weights,
            tc.tile_pool(name="acts", bufs=1) as acts,
            tc.tile_pool(name="ybuf", bufs=1) as ybuf,
            tc.tile_pool(name="stream", bufs=8) as stream,
            tc.tile_pool(name="work", bufs=2) as work,
            tc.tile_pool(name="lrec", bufs=2) as lrec,
            tc.tile_pool(name="expp", bufs=4) as expp,
            tc.tile_pool(name="outp", bufs=3) as outp,
            tc.tile_pool(name="psum", bufs=7, space="PSUM") as psum,
            tc.tile_pool(name="psumv", bufs=1, space="PSUM") as psumv,
        ):
            # ---------- constants ----------
            cos_sb = consts.tile([128, T], F32, tag="cos")
            nc.sync.dma_start(out=cos_sb, in_=cos2[:, :])
            sin_sb = consts.tile([128, T], F32, tag="sin")
            nc.sync.dma_start(out=sin_sb, in_=sin2[:, :])
            mask_sb = consts.tile([128, 4 * 512], BF16, tag="mask")
            nc.sync.dma_start(out=mask_sb, in_=masks[:, :])
            ones_col = consts.tile([128, 1], BF16, tag="onesc")
            nc.vector.memset(ones_col, 1.0)
            ones_row = consts.tile([1, 128], BF16, tag="onesr")
            nc.vector.memset(ones_row, 1.0)
            ident = consts.tile([128, 128], BF16, tag="ident")
            make_identity(nc, ident)

            # ---------- resident weights (wo loaded later, after attn(0)) ----------
            wq_r = wq_p.rearrange("p (n m) -> p n m", n=NKT)
            wq_sb = weights.tile([128, NKT, NREP * HD], FP16, tag="wq")
            for s in range(4):
                nc.sync.dma_start(
                    out=wq_sb[:, 8 * s:8 * (s + 1), :],
                    in_=wq_r[:, 8 * s:8 * (s + 1), :],
                )
            wk_sb = weights.tile([128, NKT, HD], FP16, tag="wk")
            nc.sync.dma_start(
                out=wk_sb, in_=wk_p.rearrange("p (n m) -> p n m", n=NKT)
            )
            wv_sb = weights.tile([128, NKT, HD], FP16, tag="wv")
            nc.sync.dma_start(
                out=wv_sb, in_=wv_p.rearrange("p (n m) -> p n m", n=NKT)
            )
            wo_sb = weights.tile([128, NKT, NREP * HD], FP16, tag="wo")

            # activations that live through the attention phase
            qT_sb = acts.tile([128, NREP, T], FP16, tag="qt")
            kT_sb = acts.tile([128, T], FP16, tag="kt")
            vkd_sb = acts.tile([128, NTT, HD], BF16, tag="vkd")

            def proj_chunk(qc):
                """QKV projections + rope for token chunk qc."""
                cs = slice(512 * qc, 512 * (qc + 1))
                q_ps = [
                    psum.tile([128, 512], F32, tag="bank", name=f"qps{qc}_{h}")
                    for h in range(NREP)
                ]
                k_ps = psum.tile([128, 512], F32, tag="bank", name=f"kps{qc}")
                v_ps = psum.tile([128, 512], F32, tag="bank", name=f"vps{qc}")
                for kt in range(NKT):
                    xt = stream.tile([128, 512], FP16, tag="xt")
                    nc.sync.dma_start(
                        out=xt, in_=xT[128 * kt:128 * (kt + 1), cs]
                    )
                    st = kt == 0
                    sp = kt == NKT - 1
                    for h in range(NREP):
                        nc.tensor.matmul(
                            q_ps[h],
                            lhsT=wq_sb[:, kt, 128 * h:128 * (h + 1)],
                            rhs=xt,
                            start=st,
                            stop=sp,
                        )
                    nc.tensor.matmul(
                        k_ps, lhsT=wk_sb[:, kt, :], rhs=xt, start=st, stop=sp
                    )
                    nc.tensor.matmul(
                        v_ps, lhsT=wv_sb[:, kt, :], rhs=xt, start=st, stop=sp
                    )

                # rope: k first (unblocks h=0 scores), then the 4 q heads
                for h in [NREP, 0, 1, 2, 3]:
                    p = q_ps[h] if h < NREP else k_ps
                    dst = qT_sb[:, h, cs] if h < NREP else kT_sb[:, cs]
                    sw = work.tile([128, 512], F32, tag="sw")
                    nc.scalar.copy(sw[0:64, :], p[64:128, :])
                    nc.scalar.copy(sw[64:128, :], p[0:64, :])
                    rtmp = work.tile([128, 512], F32, tag="ropetmp")
                    # dst = p * cos + sw * (+-sin), fp16 conversion on the add
                    nc.vector.tensor_mul(rtmp, p, cos_sb[:, cs])
                    nc.vector.tensor_mul(sw, sw, sin_sb[:, cs])
                    nc.vector.tensor_add(dst, rtmp, sw)

                # v computed in [hd, T] layout; transpose 128x128 blocks to [k, hd]
                v_sb = work.tile([128, 512], BF16, tag="vsb")
                nc.scalar.copy(v_sb, v_ps)
                for s in range(4):
                    vt_ps = psumv.tile(
                        [128, 128], BF16, tag="vtbank", name=f"vt{qc}_{s}"
                    )
                    nc.tensor.transpose(
                        vt_ps, v_sb[:, 128 * s:128 * (s + 1)], ident
                    )
                    nc.scalar.copy(vkd_sb[:, 4 * qc + s, :], vt_ps)

            def attn_chunk(qc):
                """Causal attention for all 4 heads on chunk qc."""
                cs = slice(512 * qc, 512 * (qc + 1))
                nkt = 4 * qc + 4  # causal: k tiles 0 .. 4*qc+3
                for h in range(NREP):
                    yT_ps = psum.tile(
                        [128, 512], F32, tag="bank", name=f"yps{qc}_{h}"
                    )
                    l_ps = psum.tile([128, 512], F32, tag="bank", name=f"l{qc}{h}")
                    for kt in range(nkt):
                        sT_ps = psum.tile(
                            [128, 512], F32, tag="bank", name=f"sps{qc}_{h}_{kt}"
                        )
                        nc.tensor.matmul(
                            sT_ps,
                            lhsT=kT_sb[:, 128 * kt:128 * (kt + 1)],
                            rhs=qT_sb[:, h, cs],
                            start=True,
                            stop=True,
                        )
                        e_sb = expp.tile([128, 512], BF16, tag="exp")
                        nc.scalar.activation(
                            e_sb, sT_ps, mybir.ActivationFunctionType.Exp,
                            scale=SCALE,
                        )
                        d = kt - 4 * qc
                        if d >= 0:  # diagonal block: zero the k > q half
                            nc.vector.tensor_mul(
                                e_sb, e_sb, mask_sb[:, 512 * d:512 * (d + 1)]
                            )
                        nc.tensor.matmul(
                            l_ps[0:1, :], lhsT=ones_col[:, 0:1], rhs=e_sb,
                            start=(kt == 0), stop=(kt == nkt - 1),
                        )
                        nc.tensor.matmul(
                            yT_ps,
                            lhsT=vkd_sb[:, kt, :],
                            rhs=e_sb,
                            start=(kt == 0),
                            stop=(kt == nkt - 1),
                        )
                    # softmax denominator -> reciprocal -> broadcast to 128 parts
                    recip = lrec.tile([1, 512], F32, tag="recip")
                    nc.vector.reciprocal_approx_fast(recip, l_ps[0:1, :])
                    recip_bf = lrec.tile([1, 512], BF16, tag="recipbf")
                    nc.scalar.copy(recip_bf, recip)
                    bc_ps = psum.tile([128, 512], F32, tag="bank", name=f"b{qc}{h}")
                    nc.tensor.matmul(
                        bc_ps, lhsT=ones_row[0:1, :], rhs=recip_bf[0:1, :],
                        start=True, stop=True,
                    )
                    bc_sb = work.tile([128, 512], F32, tag="bc")
                    nc.scalar.copy(bc_sb, bc_ps)
                    yn_sb = work.tile([128, 512], FP16, tag="yn")
                    nc.vector.tensor_mul(yn_sb, yT_ps, bc_sb)
                    nc.sync.dma_start(
                        out=y_in[qc][128 * h:128 * (h + 1), :], in_=yn_sb
                    )
                nc.gpsimd.collective_compute(
                    "AllGather",
                    mybir.AluOpType.bypass,
                    ins=[y_in[qc][:, :]],
                    outs=[y_all[qc][:, :]],
                    replica_groups=[list(range(N_CORES))],
                )

            def wo_chunk(qc):
                """out rows for chunk qc: needs y_all[qc] (all cores' heads)."""
                y_sb = ybuf.tile([128, NKT, 512], FP16, tag="ysb")
                y_r = y_all[qc].rearrange("(n p) m -> p n m", p=128)
                for s in range(4):
                    nc.sync.dma_start(
                        out=y_sb[:, 8 * s:8 * (s + 1), :],
                        in_=y_r[:, 8 * s:8 * (s + 1), :],
                    )
                for tt in range(4 * qc, 4 * qc + 4):
                    to = 128 * tt - 512 * qc
                    o_ps = psum.tile([128, 512], F32, tag="bank", name=f"o{tt}")
                    for kt in range(NKT):
                        nc.tensor.matmul(
                            o_ps,
                            lhsT=y_sb[:, kt, to:to + 128],
                            rhs=wo_sb[:, kt, :],
                            start=(kt == 0),
                            stop=(kt == NKT - 1),
                        )
                    o_sb = outp.tile([128, 512], F32, tag="osb")
                    nc.scalar.copy(o_sb, o_ps)
                    nc.sync.dma_start(
                        out=out[128 * tt:128 * (tt + 1), :], in_=o_sb
                    )

            # ---------- chunk-pipelined schedule ----------
            # wo(qc) is issued two chunks late so the AllGather latency is
            # covered by proj/attn of the following chunks.
            proj_chunk(0)
            attn_chunk(0)
            wo_r = wo_p.rearrange("p (n m) -> p n m", n=NKT)
            for s in range(4):
                nc.sync.dma_start(
                    out=wo_sb[:, 8 * s:8 * (s + 1), :],
                    in_=wo_r[:, 8 * s:8 * (s + 1), :],
                )
            proj_chunk(1)
            attn_chunk(1)
            proj_chunk(2)
            attn_chunk(2)
            wo_chunk(0)
            proj_chunk(3)
            attn_chunk(3)
            wo_chunk(1)
            wo_chunk(2)
            wo_chunk(3)

    nc.compile()
    _cached["nc"] = nc
    return nc


def _build_in_maps(inputs):
    return _shard_inputs(**inputs)


def _pack_w(wT, m):
    """[DIM, m] -> [128, NKT*m] with each partition's rows DRAM-contiguous."""
    return np.ascontiguousarray(
        wT.reshape(NKT, 128, m).transpose(1, 0, 2).reshape(128, NKT * m)
    )


def _shard_inputs(x, cos, sin, wq, wk, wv, wo, start_pos):
    import ml_dtypes

    bf16 = ml_dtypes.bfloat16
    x = np.asarray(x, dtype=np.float32)
    cos = np.asarray(cos, dtype=np.float32)
    sin = np.asarray(sin, dtype=np.float32)
    wq = np.asarray(wq, dtype=np.float32)
    wk = np.asarray(wk, dtype=np.float32)
    wv = np.asarray(wv, dtype=np.float32)
    wo = np.asarray(wo, dtype=np.float32)
    sp = int(start_pos)

    xT = np.ascontiguousarray(x[0].T).astype(np.float16)  # (DIM, T)
    cosT = np.ascontiguousarray(cos[sp:sp + T].T)  # (64, T)
    sinT = np.ascontiguousarray(sin[sp:sp + T].T)
    cos2 = np.concatenate([cosT, cosT], axis=0)  # (128, T)
    sin2 = np.concatenate([-sinT, sinT], axis=0)  # rotate-half signs folded in

    kk = np.arange(128)[:, None]
    qq = np.arange(512)[None, :]
    masks = np.concatenate(
        [(kk + 128 * d <= qq).astype(np.float32) for d in range(4)], axis=1
    ).astype(bf16)  # (128, 2048)

    in_maps = []
    for c in range(N_CORES):
        qrows = slice(NREP * HD * c, NREP * HD * (c + 1))
        krows = slice(HD * c, HD * (c + 1))
        in_maps.append({
            "xT": xT,
            "cos2": cos2,
            "sin2": sin2,
            "masks": masks,
            "wq_p": _pack_w(wq[qrows, :].T.astype(np.float16), NREP * HD),
            "wk_p": _pack_w(wk[krows, :].T.astype(np.float16), HD),
            "wv_p": _pack_w(wv[krows, :].T.astype(np.float16), HD),
            "wo_p": _pack_w(wo[qrows, :].T.astype(np.float16), NREP * HD),
        })
    return in_maps


def kernel(x, cos, sin, wq, wk, wv, wo, start_pos):
    in_maps = _shard_inputs(x, cos, sin, wq, wk, wv, wo, start_pos)
    nc = _build_kernel()
    res = run_bass_kernel_spmd(nc, in_maps, core_ids=list(range(N_CORES)))
    out = np.concatenate([res.results[c]["out"] for c in range(N_CORES)], axis=1)
    return out.reshape(1, T, DIM).astype(np.float32)
